# revision 14
# baseline (speedup 1.0000x reference)
"""Causal multi-head attention (B=2, S=2048, D=1024, H=16, Dh=64) on 8 trn2 cores.

Sharding: head-parallel. Core c owns heads {2c, 2c+1} (a 128-wide slice of the
QKV output dim) and computes attention for all 4096 token rows for those heads.
The context slabs are redistributed with an on-device AllToAll so core c ends up
with the full 1024-d context for token rows [c*512, (c+1)*512), then applies the
output projection for those rows. Matmuls run in bf16 (validated ~3.5e-3 rel err).
"""

import sys

for _p in ("/root/.axon_site/_ro/trn_rl_repo", "/opt/trn_rl_repo"):
    if _p not in sys.path:
        sys.path.insert(0, _p)

import numpy as np
import ml_dtypes

from contextlib import ExitStack

from concourse import bacc, bass, mybir, tile
from concourse.bass_utils import run_bass_kernel_spmd

BF16 = mybir.dt.bfloat16
F32 = mybir.dt.float32

B, S, D = 2, 2048, 1024
H, DH = 16, 64
NC = 8          # cores
R = B * S       # 4096 token rows
NCH = R // 512  # 8 column chunks of 512 (also = number of q-jobs = A2A ranks)
NIB = D // 128  # 8 input-dim blocks
NKB = 32        # global k-blocks of 128 (16 per batch)

LAST_EXEC_NS = None
LAST_RESULTS = None


def build():
    nc = bacc.Bacc(trn_type="TRN2", num_devices=NC)

    xT = nc.dram_tensor("xT", [128, NIB, NCH, 512], BF16, kind="ExternalInput")
    wq = nc.dram_tensor("wq", [128, NIB * 128], BF16, kind="ExternalInput")
    wk = nc.dram_tensor("wk", [128, NIB * 128], BF16, kind="ExternalInput")
    wv = nc.dram_tensor("wv", [128, NIB * 128], BF16, kind="ExternalInput")
    wo = nc.dram_tensor("wo", [128, NCH * 1024], BF16, kind="ExternalInput")
    bias = nc.dram_tensor("bias", [128, 1024], F32, kind="ExternalInput")
    masks = nc.dram_tensor("masks", [4, 128, 512], BF16, kind="ExternalInput")
    out = nc.dram_tensor("out", [512, 1024], F32, kind="ExternalOutput")

    with tile.TileContext(nc) as tc, ExitStack() as ctx:
        pool = ctx.enter_context(tc.tile_pool(name="persist", bufs=1))

        qt_sb = pool.tile([128, R], BF16)          # Q^T for my 2 heads
        kt_sb = pool.tile([128, R], BF16)          # K^T for my 2 heads
        v_sb = pool.tile([128, NKB * 132], BF16)   # per k-block: V_A|1|V_B|1
        wq_sb = pool.tile([128, NIB * 128], BF16)
        wk_sb = pool.tile([128, NIB * 128], BF16)
        wv_sb = pool.tile([128, NIB * 128], BF16)
        wo_sb = pool.tile([128, NCH * 1024], BF16)
        bias_sb = pool.tile([128, 1024], F32)
        mask_sb = pool.tile([128, 4 * 512], BF16)
        ctxt_sb = pool.tile([128, NCH * 512], BF16)  # post-A2A full ctx^T

        nc.sync.dma_start(wq_sb[:, :], wq[:, :])
        nc.sync.dma_start(wk_sb[:, :], wk[:, :])
        nc.sync.dma_start(wv_sb[:, :], wv[:, :])
        nc.sync.dma_start(wo_sb[:, :], wo[:, :])
        nc.sync.dma_start(bias_sb[:, :], bias[:, :])
        for j in range(4):
            nc.sync.dma_start(mask_sb[:, j * 512:(j + 1) * 512], masks[j])

        # ones columns of v_sb (col 64 and 129 of each 132-block)
        ones_ap = v_sb[:, :].rearrange("p (b t) -> p b t", t=132)[:, :, 64:130:65]
        nc.vector.memset(ones_ap, 1.0)
        ones_sb = pool.tile([128, 64], BF16)
        nc.vector.memset(ones_sb[:, :], 1.0)

        dram = ctx.enter_context(tc.tile_pool(name="dram", bufs=1, space="DRAM"))
        ctx_send = dram.tile([NCH, 128, 512], BF16)
        ctx_recv = dram.tile([NCH, 128, 512], BF16)

        # ---- Phase 1: QKV projections ----
        with tc.tile_pool(name="xt", bufs=2) as xtp, \
             tc.tile_pool(name="p1ps", bufs=2, space="PSUM") as p1ps, \
             tc.tile_pool(name="p1vps", bufs=2, space="PSUM") as p1vps:
            for g in range(NCH):
                xt = xtp.tile([128, NIB * 512], BF16)
                nc.sync.dma_start(
                    xt[:, :].rearrange("p (i c) -> p i c", c=512), xT[:, :, g, :]
                )
                qt_ps = p1ps.tile([128, 512], F32)
                kt_ps = p1ps.tile([128, 512], F32)
                for i in range(NIB):
                    nc.tensor.matmul(
                        qt_ps[:, :], wq_sb[:, i * 128:(i + 1) * 128],
                        xt[:, i * 512:(i + 1) * 512],
                        start=(i == 0), stop=(i == NIB - 1),
                    )
                for i in range(NIB):
                    nc.tensor.matmul(
                        kt_ps[:, :], wk_sb[:, i * 128:(i + 1) * 128],
                        xt[:, i * 512:(i + 1) * 512],
                        start=(i == 0), stop=(i == NIB - 1),
                    )
                nc.scalar.copy(qt_sb[:, g * 512:(g + 1) * 512], qt_ps[:, :])
                nc.vector.tensor_copy(kt_sb[:, g * 512:(g + 1) * 512], kt_ps[:, :])

                # V in [k, d] layout: lhsT = xT block, rhs = wv block
                for kb in range(4):
                    v_ps = p1vps.tile([128, 128], F32)
                    for i in range(NIB):
                        nc.tensor.matmul(
                            v_ps[:, :],
                            xt[:, i * 512 + kb * 128: i * 512 + (kb + 1) * 128],
                            wv_sb[:, i * 128:(i + 1) * 128],
                            start=(i == 0), stop=(i == NIB - 1),
                        )
                    base = (g * 4 + kb) * 132
                    dst = v_sb[:, base:base + 130].rearrange(
                        "p (h t) -> p h t", h=2
                    )[:, :, 0:64]
                    src = v_ps[:, :].rearrange("p (h t) -> p h t", h=2)
                    nc.vector.tensor_copy(dst, src)

        # ---- Phase 2: attention for my 2 heads, all 8 q-chunks ----
        with tc.tile_pool(name="sps", bufs=2, space="PSUM") as sps, \
             tc.tile_pool(name="cps", bufs=1, space="PSUM") as cps, \
             tc.tile_pool(name="ep", bufs=3) as ep, \
             tc.tile_pool(name="rp", bufs=2) as rp, \
             tc.tile_pool(name="bcp", bufs=1, space="PSUM") as bcp, \
             tc.tile_pool(name="bcs", bufs=2) as bcsp, \
             tc.tile_pool(name="slab", bufs=2) as slabp:
            for t in range(NCH):
                b, qc = t // 4, t % 4
                nkb = 4 * (qc + 1)
                ctx_a = cps.tile([65, 512], F32)
                ctx_b = cps.tile([65, 512], F32)
                for kb in range(nkb):
                    vb = b * 16 + kb
                    kc = vb * 128
                    s_a = sps.tile([128, 512], F32)
                    s_b = sps.tile([128, 512], F32)
                    nc.tensor.matmul(
                        s_a[:, :], kt_sb[0:64, kc:kc + 128],
                        qt_sb[0:64, t * 512:(t + 1) * 512],
                    )
                    nc.tensor.matmul(
                        s_b[:, :], kt_sb[64:128, kc:kc + 128],
                        qt_sb[64:128, t * 512:(t + 1) * 512],
                    )
                    e_a = ep.tile([128, 512], BF16)
                    e_b = ep.tile([128, 512], BF16)
                    nc.scalar.activation(
                        e_a[:, :], s_a[:, :], mybir.ActivationFunctionType.Exp,
                        scale=0.125,
                    )
                    nc.scalar.activation(
                        e_b[:, :], s_b[:, :], mybir.ActivationFunctionType.Exp,
                        scale=0.125,
                    )
                    j = kb - 4 * qc
                    if j >= 0:
                        m = mask_sb[:, j * 512:(j + 1) * 512]
                        nc.vector.tensor_mul(e_a[:, :], e_a[:, :], m)
                        nc.vector.tensor_mul(e_b[:, :], e_b[:, :], m)
                    vbase = vb * 132
                    nc.tensor.matmul(
                        ctx_a[:, :], v_sb[:, vbase:vbase + 65], e_a[:, :],
                        start=(kb == 0), stop=(kb == nkb - 1),
                    )
                    nc.tensor.matmul(
                        ctx_b[:, :], v_sb[:, vbase + 65:vbase + 130], e_b[:, :],
                        start=(kb == 0), stop=(kb == nkb - 1),
                    )
                # normalize: row 64 of ctx psum = sum(e) per q
                rec_a = rp.tile([65, 512], BF16)
                rec_b = rp.tile([65, 512], BF16)
                with nc.allow_low_precision(reason="softmax denom recip in bf16"):
                    nc.vector.reciprocal(rec_a[64:65, :], ctx_a[64:65, :])
                    nc.vector.reciprocal(rec_b[64:65, :], ctx_b[64:65, :])
                # broadcast recip row across 64 partitions via ones-matmul
                bc_a = bcp.tile([64, 512], F32)
                bc_b = bcp.tile([64, 512], F32)
                nc.tensor.matmul(bc_a[:, :], ones_sb[64:65, :], rec_a[64:65, :])
                nc.tensor.matmul(bc_b[:, :], ones_sb[64:65, :], rec_b[64:65, :])
                # DVE can read only one PSUM operand: stage bc in SBUF via ACT
                bcs_a = bcsp.tile([64, 512], F32)
                bcs_b = bcsp.tile([64, 512], F32)
                nc.scalar.copy(bcs_a[:, :], bc_a[:, :])
                nc.scalar.copy(bcs_b[:, :], bc_b[:, :])
                sl_a = slabp.tile([64, 512], BF16)
                sl_b = slabp.tile([64, 512], BF16)
                nc.vector.tensor_mul(sl_a[:, :], ctx_a[0:64, :], bcs_a[:, :])
                nc.vector.tensor_mul(sl_b[:, :], ctx_b[0:64, :], bcs_b[:, :])
                nc.sync.dma_start(ctx_send[t, 0:64, :], sl_a[:, :])
                nc.sync.dma_start(ctx_send[t, 64:128, :], sl_b[:, :])

        # ---- Phase 3: AllToAll redistributes ctx slabs ----
        nc.gpsimd.collective_compute(
            "AllToAll",
            mybir.AluOpType.bypass,
            replica_groups=[list(range(NC))],
            ins=[ctx_send.opt()],
            outs=[ctx_recv.opt()],
        )

        # ---- Phase 4: output projection for my 512 rows ----
        for t2 in range(NCH):
            nc.sync.dma_start(ctxt_sb[:, t2 * 512:(t2 + 1) * 512], ctx_recv[t2])
        with tc.tile_pool(name="ops", bufs=2, space="PSUM") as ops, \
             tc.tile_pool(name="osb", bufs=2) as osb:
            for rb in range(4):
                po0 = ops.tile([128, 512], F32)
                po1 = ops.tile([128, 512], F32)
                for t2 in range(NCH):
                    lhsT = ctxt_sb[:, t2 * 512 + rb * 128: t2 * 512 + rb * 128 + 128]
                    nc.tensor.matmul(
                        po0[:, :], lhsT, wo_sb[:, t2 * 1024: t2 * 1024 + 512],
                        start=(t2 == 0), stop=(t2 == NCH - 1),
                    )
                    nc.tensor.matmul(
                        po1[:, :], lhsT, wo_sb[:, t2 * 1024 + 512:(t2 + 1) * 1024],
                        start=(t2 == 0), stop=(t2 == NCH - 1),
                    )
                o_sb = osb.tile([128, 1024], F32)
                nc.vector.tensor_add(o_sb[:, 0:512], po0[:, :], bias_sb[:, 0:512])
                nc.vector.tensor_add(o_sb[:, 512:1024], po1[:, :], bias_sb[:, 512:1024])
                nc.sync.dma_start(out[rb * 128:(rb + 1) * 128, :], o_sb[:, :])

    nc.finalize()
    return nc


def prep_inputs(x, w_query, w_key, w_value, w_out, b_out):
    bf = ml_dtypes.bfloat16
    xt = np.ascontiguousarray(
        x.reshape(R, D).T.reshape(NIB, 128, NCH, 512).transpose(1, 0, 2, 3)
    ).astype(bf)
    wo_p = np.ascontiguousarray(
        w_out.T.reshape(NCH, 128, 1024).transpose(1, 0, 2).reshape(128, NCH * 1024)
    ).astype(bf)
    bias_p = np.ascontiguousarray(np.broadcast_to(b_out, (128, 1024))).astype(np.float32)
    p_idx = np.arange(128)[:, None]
    f_idx = np.arange(512)[None, :]
    masks_p = np.stack(
        [(f_idx >= p_idx + j * 128).astype(bf) for j in range(4)]
    )

    def wslice(w, c):
        blk = w[c * 128:(c + 1) * 128, :].T  # [1024 in, 128 out]
        return np.ascontiguousarray(
            blk.reshape(NIB, 128, 128).transpose(1, 0, 2).reshape(128, NIB * 128)
        ).astype(bf)

    in_maps = []
    for c in range(NC):
        in_maps.append({
            "xT": xt,
            "wq": wslice(w_query, c),
            "wk": wslice(w_key, c),
            "wv": wslice(w_value, c),
            "wo": wo_p,
            "bias": bias_p,
            "masks": masks_p,
        })
    return in_maps


def kernel(x, w_query, w_key, w_value, w_out, b_out):
    global LAST_EXEC_NS, LAST_RESULTS
    x = np.asarray(x, dtype=np.float32)
    w_query = np.asarray(w_query, dtype=np.float32)
    w_key = np.asarray(w_key, dtype=np.float32)
    w_value = np.asarray(w_value, dtype=np.float32)
    w_out = np.asarray(w_out, dtype=np.float32)
    b_out = np.asarray(b_out, dtype=np.float32)

    nc = build()
    in_maps = prep_inputs(x, w_query, w_key, w_value, w_out, b_out)
    try:
        br = run_bass_kernel_spmd(nc, in_maps, list(range(NC)), trace=True)
    except Exception:
        br = run_bass_kernel_spmd(nc, in_maps, list(range(NC)), trace=False)
    LAST_EXEC_NS = br.exec_time_ns
    LAST_RESULTS = br

    out = np.empty((R, D), dtype=np.float32)
    for c in range(NC):
        out[c * 512:(c + 1) * 512, :] = br.results[c]["out"]
    return out.reshape(B, S, D)


if __name__ == "__main__":
    rng = np.random.default_rng(0)
    ins = {
        "x": rng.standard_normal((B, S, D), dtype=np.float32),
        "w_query": rng.standard_normal((D, D), dtype=np.float32) * 0.03,
        "w_key": rng.standard_normal((D, D), dtype=np.float32) * 0.03,
        "w_value": rng.standard_normal((D, D), dtype=np.float32) * 0.03,
        "w_out": rng.standard_normal((D, D), dtype=np.float32) * 0.03,
        "b_out": rng.standard_normal((D,), dtype=np.float32) * 0.03,
    }
    y = kernel(**ins)
    print("out", y.shape, y.dtype, "exec_ns", LAST_EXEC_NS)


# revision 30
# speedup vs baseline: 1.1970x; 1.1970x over previous
"""Causal multi-head attention (B=2, S=2048, D=1024, H=16, Dh=64) on 8 trn2 cores.

Sharding: head-parallel. Core c owns heads {2c, 2c+1} (a 128-wide slice of the
QKV output dim) and computes attention for all 4096 token rows for those heads.
The context slabs are redistributed with an on-device AllToAll so core c ends up
with the full 1024-d context for token rows [c*512, (c+1)*512), then applies the
output projection for those rows. Matmuls run in bf16 (validated ~3.5e-3 rel err).
"""

import sys

for _p in ("/root/.axon_site/_ro/trn_rl_repo", "/opt/trn_rl_repo"):
    if _p not in sys.path:
        sys.path.insert(0, _p)

import numpy as np
import ml_dtypes

from contextlib import ExitStack

from concourse import bacc, bass, mybir, tile
from concourse.bass_utils import run_bass_kernel_spmd

BF16 = mybir.dt.bfloat16
F32 = mybir.dt.float32

B, S, D = 2, 2048, 1024
H, DH = 16, 64
NC = 8          # cores
R = B * S       # 4096 token rows
NCH = R // 512  # 8 column chunks of 512 (also = number of q-jobs = A2A ranks)
NIB = D // 128  # 8 input-dim blocks
NKB = 32        # global k-blocks of 128 (16 per batch)

LAST_EXEC_NS = None
LAST_RESULTS = None


def build():
    nc = bacc.Bacc(trn_type="TRN2", num_devices=NC)

    xT = nc.dram_tensor("xT", [128, NIB, NCH, 512], BF16, kind="ExternalInput")
    wq = nc.dram_tensor("wq", [128, NIB * 128], BF16, kind="ExternalInput")
    wk = nc.dram_tensor("wk", [128, NIB * 128], BF16, kind="ExternalInput")
    wv = nc.dram_tensor("wv", [128, NIB * 128], BF16, kind="ExternalInput")
    wo = nc.dram_tensor("wo", [128, NCH * 1024], BF16, kind="ExternalInput")
    bias = nc.dram_tensor("bias", [128, 1024], F32, kind="ExternalInput")
    masks = nc.dram_tensor("masks", [128, 128], BF16, kind="ExternalInput")
    out = nc.dram_tensor("out", [512, 1024], F32, kind="ExternalOutput")

    with tile.TileContext(nc) as tc, ExitStack() as ctx:
        pool = ctx.enter_context(tc.tile_pool(name="persist", bufs=1))

        qt_sb = pool.tile([128, R], BF16)          # Q^T for my 2 heads
        kt_sb = pool.tile([128, R], BF16)          # K^T for my 2 heads
        v_sb = pool.tile([128, NKB * 132], BF16)   # per k-block: V_A|1|V_B|1
        wq_sb = pool.tile([128, NIB * 128], BF16)
        wk_sb = pool.tile([128, NIB * 128], BF16)
        wv_sb = pool.tile([128, NIB * 128], BF16)
        wo_sb = pool.tile([128, NCH * 1024], BF16)
        bias_sb = pool.tile([128, 1024], F32)
        mask_sb = pool.tile([128, 128], BF16)
        ctxt_sb = pool.tile([128, NCH * 512], BF16)  # post-A2A full ctx^T

        nc.sync.dma_start(wq_sb[:, :], wq[:, :])
        nc.sync.dma_start(wk_sb[:, :], wk[:, :])
        nc.sync.dma_start(wv_sb[:, :], wv[:, :])
        nc.sync.dma_start(wo_sb[:, :], wo[:, :])
        nc.sync.dma_start(bias_sb[:, :], bias[:, :])
        nc.sync.dma_start(mask_sb[:, :], masks[:, :])

        # ones columns of v_sb (col 64 and 129 of each 132-block)
        ones_ap = v_sb[:, :].rearrange("p (b t) -> p b t", t=132)[:, :, 64:130:65]
        nc.vector.memset(ones_ap, 1.0)
        ones_sb = pool.tile([128, 64], BF16)
        nc.vector.memset(ones_sb[:, :], 1.0)

        dram = ctx.enter_context(tc.tile_pool(name="dram", bufs=1, space="DRAM"))
        ctx_send = dram.tile([NCH, 128, 512], BF16)
        ctx_recv = dram.tile([NCH, 128, 512], BF16)

        # ---- Phase 1: QKV projections ----
        with tc.tile_pool(name="xt", bufs=2) as xtp, \
             tc.tile_pool(name="p1ps", bufs=2, space="PSUM") as p1ps, \
             tc.tile_pool(name="p1vps", bufs=2, space="PSUM") as p1vps:
            for g in range(NCH):
                xt = xtp.tile([128, NIB * 512], BF16)
                nc.sync.dma_start(
                    xt[:, :].rearrange("p (i c) -> p i c", c=512), xT[:, :, g, :]
                )
                qt_ps = p1ps.tile([128, 512], F32)
                kt_ps = p1ps.tile([128, 512], F32)
                for i in range(NIB):
                    nc.tensor.matmul(
                        qt_ps[:, :], wq_sb[:, i * 128:(i + 1) * 128],
                        xt[:, i * 512:(i + 1) * 512],
                        start=(i == 0), stop=(i == NIB - 1),
                    )
                for i in range(NIB):
                    nc.tensor.matmul(
                        kt_ps[:, :], wk_sb[:, i * 128:(i + 1) * 128],
                        xt[:, i * 512:(i + 1) * 512],
                        start=(i == 0), stop=(i == NIB - 1),
                    )
                nc.scalar.copy(qt_sb[:, g * 512:(g + 1) * 512], qt_ps[:, :])
                nc.vector.tensor_copy(kt_sb[:, g * 512:(g + 1) * 512], kt_ps[:, :])

                # V in [k, d] layout: lhsT = xT block, rhs = wv block
                for kb in range(4):
                    v_ps = p1vps.tile([128, 128], F32)
                    for i in range(NIB):
                        nc.tensor.matmul(
                            v_ps[:, :],
                            xt[:, i * 512 + kb * 128: i * 512 + (kb + 1) * 128],
                            wv_sb[:, i * 128:(i + 1) * 128],
                            start=(i == 0), stop=(i == NIB - 1),
                        )
                    base = (g * 4 + kb) * 132
                    dst = v_sb[:, base:base + 130].rearrange(
                        "p (h t) -> p h t", h=2
                    )[:, :, 0:64]
                    src = v_ps[:, :].rearrange("p (h t) -> p h t", h=2)
                    nc.vector.tensor_copy(dst, src)

        # ---- Phase 2: attention for my 2 heads, all 8 q-chunks ----
        with tc.tile_pool(name="sps", bufs=2, space="PSUM") as sps, \
             tc.tile_pool(name="cps", bufs=1, space="PSUM") as cps, \
             tc.tile_pool(name="ep", bufs=3) as ep, \
             tc.tile_pool(name="csb", bufs=2) as csbp, \
             tc.tile_pool(name="rp", bufs=2) as rp, \
             tc.tile_pool(name="bcp", bufs=1, space="PSUM") as bcp, \
             tc.tile_pool(name="slab", bufs=2) as slabp:
            for t in range(NCH):
                b, qc = t // 4, t % 4
                nkb = 4 * (qc + 1)
                ctx_a = cps.tile([65, 512], F32)
                ctx_b = cps.tile([65, 512], F32)
                for kb in range(nkb):
                    vb = b * 16 + kb
                    kc = vb * 128
                    j = kb - 4 * qc
                    lo = max(j, 0) * 128  # q cols below lo are fully masked
                    s_a = sps.tile([128, 512], F32)
                    s_b = sps.tile([128, 512], F32)
                    nc.tensor.matmul(
                        s_a[:, lo:], kt_sb[0:64, kc:kc + 128],
                        qt_sb[0:64, t * 512 + lo:(t + 1) * 512],
                    )
                    nc.tensor.matmul(
                        s_b[:, lo:], kt_sb[64:128, kc:kc + 128],
                        qt_sb[64:128, t * 512 + lo:(t + 1) * 512],
                    )
                    e_a = ep.tile([128, 512], BF16)
                    e_b = ep.tile([128, 512], BF16)
                    nc.scalar.activation(
                        e_a[:, lo:], s_a[:, lo:], mybir.ActivationFunctionType.Exp,
                        scale=0.125,
                    )
                    nc.scalar.activation(
                        e_b[:, lo:], s_b[:, lo:], mybir.ActivationFunctionType.Exp,
                        scale=0.125,
                    )
                    if j >= 0:
                        # only the 128-wide diagonal strip needs the triangle mask
                        nc.vector.tensor_mul(
                            e_a[:, lo:lo + 128], e_a[:, lo:lo + 128], mask_sb[:, :]
                        )
                        nc.vector.tensor_mul(
                            e_b[:, lo:lo + 128], e_b[:, lo:lo + 128], mask_sb[:, :]
                        )
                    vbase = vb * 132
                    nc.tensor.matmul(
                        ctx_a[:, lo:], v_sb[:, vbase:vbase + 65], e_a[:, lo:],
                        start=(kb == 0), stop=(kb == nkb - 1),
                    )
                    nc.tensor.matmul(
                        ctx_b[:, lo:], v_sb[:, vbase + 65:vbase + 130], e_b[:, lo:],
                        start=(kb == 0), stop=(kb == nkb - 1),
                    )
                # stage ctx (+denominator row 64) to SBUF, freeing the psum bank
                cs_a = csbp.tile([65, 512], F32)
                cs_b = csbp.tile([65, 512], F32)
                nc.scalar.copy(cs_a[:, :], ctx_a[:, :])
                nc.scalar.copy(cs_b[:, :], ctx_b[:, :])
                rec_a = rp.tile([65, 512], BF16)
                rec_b = rp.tile([65, 512], BF16)
                with nc.allow_low_precision(reason="softmax denom recip in bf16"):
                    nc.vector.reciprocal(rec_a[64:65, :], cs_a[64:65, :])
                    nc.vector.reciprocal(rec_b[64:65, :], cs_b[64:65, :])
                # broadcast recip row across 64 partitions via ones-matmul
                bc_a = bcp.tile([64, 512], F32)
                bc_b = bcp.tile([64, 512], F32)
                nc.tensor.matmul(bc_a[:, :], ones_sb[64:65, :], rec_a[64:65, :])
                nc.tensor.matmul(bc_b[:, :], ones_sb[64:65, :], rec_b[64:65, :])
                sl_a = slabp.tile([64, 512], BF16)
                sl_b = slabp.tile([64, 512], BF16)
                nc.vector.tensor_mul(sl_a[:, :], bc_a[:, :], cs_a[0:64, :])
                nc.vector.tensor_mul(sl_b[:, :], bc_b[:, :], cs_b[0:64, :])
                nc.sync.dma_start(ctx_send[t, 0:64, :], sl_a[:, :])
                nc.sync.dma_start(ctx_send[t, 64:128, :], sl_b[:, :])

        # ---- Phase 3: AllToAll redistributes ctx slabs ----
        nc.gpsimd.collective_compute(
            "AllToAll",
            mybir.AluOpType.bypass,
            replica_groups=[list(range(NC))],
            ins=[ctx_send.opt()],
            outs=[ctx_recv.opt()],
        )

        # ---- Phase 4: output projection for my 512 rows ----
        for t2 in range(NCH):
            nc.sync.dma_start(ctxt_sb[:, t2 * 512:(t2 + 1) * 512], ctx_recv[t2])
        with tc.tile_pool(name="ops", bufs=2, space="PSUM") as ops, \
             tc.tile_pool(name="osb", bufs=2) as osb:
            for rb in range(4):
                po0 = ops.tile([128, 512], F32)
                po1 = ops.tile([128, 512], F32)
                for t2 in range(NCH):
                    lhsT = ctxt_sb[:, t2 * 512 + rb * 128: t2 * 512 + rb * 128 + 128]
                    nc.tensor.matmul(
                        po0[:, :], lhsT, wo_sb[:, t2 * 1024: t2 * 1024 + 512],
                        start=(t2 == 0), stop=(t2 == NCH - 1),
                    )
                    nc.tensor.matmul(
                        po1[:, :], lhsT, wo_sb[:, t2 * 1024 + 512:(t2 + 1) * 1024],
                        start=(t2 == 0), stop=(t2 == NCH - 1),
                    )
                o_sb = osb.tile([128, 1024], F32)
                nc.vector.tensor_add(o_sb[:, 0:512], po0[:, :], bias_sb[:, 0:512])
                nc.vector.tensor_add(o_sb[:, 512:1024], po1[:, :], bias_sb[:, 512:1024])
                nc.sync.dma_start(out[rb * 128:(rb + 1) * 128, :], o_sb[:, :])

    nc.finalize()
    return nc


def prep_inputs(x, w_query, w_key, w_value, w_out, b_out):
    bf = ml_dtypes.bfloat16
    xt = np.ascontiguousarray(
        x.reshape(R, D).T.reshape(NIB, 128, NCH, 512).transpose(1, 0, 2, 3)
    ).astype(bf)
    wo_p = np.ascontiguousarray(
        w_out.T.reshape(NCH, 128, 1024).transpose(1, 0, 2).reshape(128, NCH * 1024)
    ).astype(bf)
    bias_p = np.ascontiguousarray(np.broadcast_to(b_out, (128, 1024))).astype(np.float32)
    masks_p = (np.arange(128)[None, :] >= np.arange(128)[:, None]).astype(bf)

    def wslice(w, c):
        blk = w[c * 128:(c + 1) * 128, :].T  # [1024 in, 128 out]
        return np.ascontiguousarray(
            blk.reshape(NIB, 128, 128).transpose(1, 0, 2).reshape(128, NIB * 128)
        ).astype(bf)

    in_maps = []
    for c in range(NC):
        in_maps.append({
            "xT": xt,
            "wq": wslice(w_query, c),
            "wk": wslice(w_key, c),
            "wv": wslice(w_value, c),
            "wo": wo_p,
            "bias": bias_p,
            "masks": masks_p,
        })
    return in_maps


def kernel(x, w_query, w_key, w_value, w_out, b_out):
    global LAST_EXEC_NS, LAST_RESULTS
    x = np.asarray(x, dtype=np.float32)
    w_query = np.asarray(w_query, dtype=np.float32)
    w_key = np.asarray(w_key, dtype=np.float32)
    w_value = np.asarray(w_value, dtype=np.float32)
    w_out = np.asarray(w_out, dtype=np.float32)
    b_out = np.asarray(b_out, dtype=np.float32)

    nc = build()
    in_maps = prep_inputs(x, w_query, w_key, w_value, w_out, b_out)
    try:
        br = run_bass_kernel_spmd(nc, in_maps, list(range(NC)), trace=True)
    except Exception:
        br = run_bass_kernel_spmd(nc, in_maps, list(range(NC)), trace=False)
    LAST_EXEC_NS = br.exec_time_ns
    LAST_RESULTS = br

    out = np.empty((R, D), dtype=np.float32)
    for c in range(NC):
        out[c * 512:(c + 1) * 512, :] = br.results[c]["out"]
    return out.reshape(B, S, D)


if __name__ == "__main__":
    rng = np.random.default_rng(0)
    ins = {
        "x": rng.standard_normal((B, S, D), dtype=np.float32),
        "w_query": rng.standard_normal((D, D), dtype=np.float32) * 0.03,
        "w_key": rng.standard_normal((D, D), dtype=np.float32) * 0.03,
        "w_value": rng.standard_normal((D, D), dtype=np.float32) * 0.03,
        "w_out": rng.standard_normal((D, D), dtype=np.float32) * 0.03,
        "b_out": rng.standard_normal((D,), dtype=np.float32) * 0.03,
    }
    y = kernel(**ins)
    print("out", y.shape, y.dtype, "exec_ns", LAST_EXEC_NS)


# revision 34
# speedup vs baseline: 1.2961x; 1.0827x over previous
"""Causal multi-head attention (B=2, S=2048, D=1024, H=16, Dh=64) on 8 trn2 cores.

Sharding: head-parallel. Core c owns heads {2c, 2c+1} (a 128-wide slice of the
QKV output dim) and computes attention for all 4096 token rows for those heads.
Context slabs (raw, with softmax-denominator rows appended) are redistributed
with two pipelined on-device AllToAlls so core c ends up with the full 1024-d
context for token rows [c*512, (c+1)*512). Normalization happens post-A2A with
one batched reciprocal per half, then the output projection for those rows.
Matmuls run in bf16 (validated ~3.5e-3 rel err).
"""

import sys

for _p in ("/root/.axon_site/_ro/trn_rl_repo", "/opt/trn_rl_repo"):
    if _p not in sys.path:
        sys.path.insert(0, _p)

import numpy as np
import ml_dtypes

from contextlib import ExitStack

from concourse import bacc, bass, mybir, tile
from concourse.bass_utils import run_bass_kernel_spmd

BF16 = mybir.dt.bfloat16
F32 = mybir.dt.float32

B, S, D = 2, 2048, 1024
H, DH = 16, 64
NC = 8          # cores
R = B * S       # 4096 token rows
NCH = R // 512  # 8 column chunks of 512 (also = A2A ranks)
NIB = D // 128  # 8 input-dim blocks
NKB = 32        # global k-blocks of 128 (16 per batch)

LAST_EXEC_NS = None
LAST_RESULTS = None


def build():
    nc = bacc.Bacc(trn_type="TRN2", num_devices=NC)

    xT = nc.dram_tensor("xT", [128, NIB, NCH, 512], BF16, kind="ExternalInput")
    wq = nc.dram_tensor("wq", [128, NIB * 128], BF16, kind="ExternalInput")
    wk = nc.dram_tensor("wk", [128, NIB * 128], BF16, kind="ExternalInput")
    wv = nc.dram_tensor("wv", [128, NIB * 128], BF16, kind="ExternalInput")
    wo = nc.dram_tensor("wo", [128, NCH * 1024], BF16, kind="ExternalInput")
    bias = nc.dram_tensor("bias", [128, 1024], F32, kind="ExternalInput")
    masks = nc.dram_tensor("masks", [128, 128], BF16, kind="ExternalInput")
    sel = nc.dram_tensor("sel", [16, NCH * 128], BF16, kind="ExternalInput")
    out = nc.dram_tensor("out", [512, 1024], F32, kind="ExternalOutput")

    with tile.TileContext(nc) as tc, ExitStack() as ctx:
        pool = ctx.enter_context(tc.tile_pool(name="persist", bufs=1))

        qt_sb = pool.tile([128, R], BF16)          # Q^T for my 2 heads
        kt_sb = pool.tile([128, R], BF16)          # K^T for my 2 heads
        v_sb = pool.tile([128, NKB * 132], BF16)   # per k-block: V_A|1|V_B|1
        wq_sb = pool.tile([128, NIB * 128], BF16)
        wk_sb = pool.tile([128, NIB * 128], BF16)
        wv_sb = pool.tile([128, NIB * 128], BF16)
        wo_sb = pool.tile([128, NCH * 1024], BF16)
        bias_sb = pool.tile([128, 1024], F32)
        mask_sb = pool.tile([128, 128], BF16)
        sel_sb = pool.tile([16, NCH * 128], BF16)
        ctxt_sb = pool.tile([128, 2 * NCH * 256], BF16)  # normalized ctx^T

        nc.sync.dma_start(wq_sb[:, :], wq[:, :])
        nc.sync.dma_start(wk_sb[:, :], wk[:, :])
        nc.sync.dma_start(wv_sb[:, :], wv[:, :])
        nc.sync.dma_start(wo_sb[:, :], wo[:, :])
        nc.sync.dma_start(bias_sb[:, :], bias[:, :])
        nc.sync.dma_start(mask_sb[:, :], masks[:, :])
        nc.sync.dma_start(sel_sb[:, :], sel[:, :])

        # ones columns of v_sb (col 64 and 129 of each 132-block)
        ones_ap = v_sb[:, :].rearrange("p (b t) -> p b t", t=132)[:, :, 64:130:65]
        nc.vector.memset(ones_ap, 1.0)

        dram = ctx.enter_context(tc.tile_pool(name="dram", bufs=1, space="DRAM"))
        ctx_send = [
            dram.tile([NCH, 130, 256], BF16, name=f"ctx_send{i}") for i in range(2)
        ]
        ctx_recv = [
            dram.tile([NCH, 130, 256], BF16, name=f"ctx_recv{i}") for i in range(2)
        ]

        # ---- Phase 1: QKV projections ----
        with tc.tile_pool(name="xt", bufs=2) as xtp, \
             tc.tile_pool(name="p1ps", bufs=2, space="PSUM") as p1ps, \
             tc.tile_pool(name="p1vps", bufs=2, space="PSUM") as p1vps:
            for g in range(NCH):
                xt = xtp.tile([128, NIB * 512], BF16)
                for i in range(NIB):
                    nc.sync.dma_start(
                        xt[:, i * 512:(i + 1) * 512], xT[:, i, g, :]
                    )
                qt_ps = p1ps.tile([128, 512], F32)
                kt_ps = p1ps.tile([128, 512], F32)
                for i in range(NIB):
                    nc.tensor.matmul(
                        qt_ps[:, :], wq_sb[:, i * 128:(i + 1) * 128],
                        xt[:, i * 512:(i + 1) * 512],
                        start=(i == 0), stop=(i == NIB - 1),
                    )
                for i in range(NIB):
                    nc.tensor.matmul(
                        kt_ps[:, :], wk_sb[:, i * 128:(i + 1) * 128],
                        xt[:, i * 512:(i + 1) * 512],
                        start=(i == 0), stop=(i == NIB - 1),
                    )
                nc.scalar.copy(qt_sb[:, g * 512:(g + 1) * 512], qt_ps[:, :])
                nc.vector.tensor_copy(kt_sb[:, g * 512:(g + 1) * 512], kt_ps[:, :])

                # V in [k, d] layout: lhsT = xT block, rhs = wv block
                for kb in range(4):
                    v_ps = p1vps.tile([128, 128], F32)
                    for i in range(NIB):
                        nc.tensor.matmul(
                            v_ps[:, :],
                            xt[:, i * 512 + kb * 128: i * 512 + (kb + 1) * 128],
                            wv_sb[:, i * 128:(i + 1) * 128],
                            start=(i == 0), stop=(i == NIB - 1),
                        )
                    base = (g * 4 + kb) * 132
                    dst = v_sb[:, base:base + 130].rearrange(
                        "p (h t) -> p h t", h=2
                    )[:, :, 0:64]
                    src = v_ps[:, :].rearrange("p (h t) -> p h t", h=2)
                    nc.vector.tensor_copy(dst, src)

        # ---- Phase 2: attention for my 2 heads, 256-row jobs, 2 halves ----
        with tc.tile_pool(name="sps", bufs=2, space="PSUM") as sps, \
             tc.tile_pool(name="cps", bufs=2, space="PSUM") as cps, \
             tc.tile_pool(name="ep", bufs=3) as ep, \
             tc.tile_pool(name="csb", bufs=2) as csbp:
            for h in range(2):
                for t in range(NCH):
                    b, qc = t // 4, t % 4
                    nkb = 4 * qc + 2 * (h + 1)
                    q0 = t * 512 + h * 256  # global column base of this job
                    ctx_a = cps.tile([65, 256], F32)
                    ctx_b = cps.tile([65, 256], F32)
                    for kb in range(nkb):
                        vb = b * 16 + kb
                        kc = vb * 128
                        j = kb - (4 * qc + 2 * h)
                        lo = max(j, 0) * 128  # q cols below lo are fully masked
                        s_a = sps.tile([128, 256], F32)
                        s_b = sps.tile([128, 256], F32)
                        nc.tensor.matmul(
                            s_a[:, lo:], kt_sb[0:64, kc:kc + 128],
                            qt_sb[0:64, q0 + lo:q0 + 256],
                        )
                        nc.tensor.matmul(
                            s_b[:, lo:], kt_sb[64:128, kc:kc + 128],
                            qt_sb[64:128, q0 + lo:q0 + 256],
                        )
                        e_a = ep.tile([128, 256], BF16)
                        e_b = ep.tile([128, 256], BF16)
                        nc.scalar.activation(
                            e_a[:, lo:], s_a[:, lo:],
                            mybir.ActivationFunctionType.Exp, scale=0.125,
                        )
                        nc.scalar.activation(
                            e_b[:, lo:], s_b[:, lo:],
                            mybir.ActivationFunctionType.Exp, scale=0.125,
                        )
                        if j >= 0:
                            # only the 128-wide diagonal strip needs the mask
                            nc.vector.tensor_mul(
                                e_a[:, lo:lo + 128], e_a[:, lo:lo + 128],
                                mask_sb[:, :],
                            )
                            nc.vector.tensor_mul(
                                e_b[:, lo:lo + 128], e_b[:, lo:lo + 128],
                                mask_sb[:, :],
                            )
                        vbase = vb * 132
                        nc.tensor.matmul(
                            ctx_a[:, lo:], v_sb[:, vbase:vbase + 65],
                            e_a[:, lo:],
                            start=(kb == 0), stop=(kb == nkb - 1),
                        )
                        nc.tensor.matmul(
                            ctx_b[:, lo:], v_sb[:, vbase + 65:vbase + 130],
                            e_b[:, lo:],
                            start=(kb == 0), stop=(kb == nkb - 1),
                        )
                    # stage raw ctx (+denominator row 64) to SBUF as bf16
                    cs_a = csbp.tile([65, 256], BF16)
                    cs_b = csbp.tile([65, 256], BF16)
                    nc.scalar.copy(cs_a[:, :], ctx_a[:, :])
                    nc.scalar.copy(cs_b[:, :], ctx_b[:, :])
                    nc.sync.dma_start(ctx_send[h][t, 0:65, :], cs_a[:, :])
                    nc.sync.dma_start(ctx_send[h][t, 65:130, :], cs_b[:, :])

                nc.gpsimd.collective_compute(
                    "AllToAll",
                    mybir.AluOpType.bypass,
                    replica_groups=[list(range(NC))],
                    ins=[ctx_send[h].opt()],
                    outs=[ctx_recv[h].opt()],
                )

        # ---- Phase 3: post-A2A normalize + output projection per half ----
        with tc.tile_pool(name="rawp", bufs=2) as rawp, \
             tc.tile_pool(name="denp", bufs=2) as denp, \
             tc.tile_pool(name="bcf", bufs=2, space="PSUM") as bcfp, \
             tc.tile_pool(name="ops", bufs=2, space="PSUM") as ops, \
             tc.tile_pool(name="osb", bufs=2) as osb:
            for h in range(2):
                rv = ctx_recv[h]
                raw = rawp.tile([128, NCH * 256], BF16)
                nc.sync.dma_start(
                    raw[0:64, :].rearrange("p (j c) -> p j c", c=256),
                    rv[:, 0:64, :].rearrange("j p c -> p j c"),
                )
                nc.sync.dma_start(
                    raw[64:128, :].rearrange("p (j c) -> p j c", c=256),
                    rv[:, 65:129, :].rearrange("j p c -> p j c"),
                )
                den = denp.tile([16, 256], BF16)
                nc.sync.dma_start(den[0:8, :], rv[:, 64, :])
                nc.sync.dma_start(den[8:16, :], rv[:, 129, :])
                rec = denp.tile([16, 256], BF16)
                with nc.allow_low_precision(reason="softmax denom recip bf16"):
                    nc.vector.reciprocal(rec[:, :], den[:, :])
                co = h * NCH * 256
                for j in range(NCH):
                    bcf = bcfp.tile([128, 256], F32)
                    nc.tensor.matmul(
                        bcf[:, :], sel_sb[:, j * 128:(j + 1) * 128], rec[:, :]
                    )
                    nc.vector.tensor_mul(
                        ctxt_sb[:, co + j * 256:co + (j + 1) * 256],
                        bcf[:, :], raw[:, j * 256:(j + 1) * 256],
                    )
                for rb in range(2):
                    po0 = ops.tile([128, 512], F32)
                    po1 = ops.tile([128, 512], F32)
                    for t2 in range(NCH):
                        lhsT = ctxt_sb[
                            :, co + t2 * 256 + rb * 128:co + t2 * 256 + rb * 128 + 128
                        ]
                        nc.tensor.matmul(
                            po0[:, :], lhsT, wo_sb[:, t2 * 1024:t2 * 1024 + 512],
                            start=(t2 == 0), stop=(t2 == NCH - 1),
                        )
                        nc.tensor.matmul(
                            po1[:, :], lhsT, wo_sb[:, t2 * 1024 + 512:(t2 + 1) * 1024],
                            start=(t2 == 0), stop=(t2 == NCH - 1),
                        )
                    o_sb = osb.tile([128, 1024], F32)
                    nc.vector.tensor_add(o_sb[:, 0:512], po0[:, :], bias_sb[:, 0:512])
                    nc.vector.tensor_add(
                        o_sb[:, 512:1024], po1[:, :], bias_sb[:, 512:1024]
                    )
                    r0 = h * 256 + rb * 128
                    nc.sync.dma_start(out[r0:r0 + 128, :], o_sb[:, :])

    nc.finalize()
    return nc


def prep_inputs(x, w_query, w_key, w_value, w_out, b_out):
    bf = ml_dtypes.bfloat16
    xt = np.ascontiguousarray(
        x.reshape(R, D).T.reshape(NIB, 128, NCH, 512).transpose(1, 0, 2, 3)
    ).astype(bf)
    wo_p = np.ascontiguousarray(
        w_out.T.reshape(NCH, 128, 1024).transpose(1, 0, 2).reshape(128, NCH * 1024)
    ).astype(bf)
    bias_p = np.ascontiguousarray(np.broadcast_to(b_out, (128, 1024))).astype(np.float32)
    masks_p = (np.arange(128)[None, :] >= np.arange(128)[:, None]).astype(bf)
    sel_p = np.zeros((16, NCH * 128), dtype=bf)
    for j in range(NCH):
        sel_p[j, j * 128:j * 128 + 64] = 1
        sel_p[8 + j, j * 128 + 64:(j + 1) * 128] = 1

    def wslice(w, c):
        blk = w[c * 128:(c + 1) * 128, :].T  # [1024 in, 128 out]
        return np.ascontiguousarray(
            blk.reshape(NIB, 128, 128).transpose(1, 0, 2).reshape(128, NIB * 128)
        ).astype(bf)

    in_maps = []
    for c in range(NC):
        in_maps.append({
            "xT": xt,
            "wq": wslice(w_query, c),
            "wk": wslice(w_key, c),
            "wv": wslice(w_value, c),
            "wo": wo_p,
            "bias": bias_p,
            "masks": masks_p,
            "sel": sel_p,
        })
    return in_maps


def kernel(x, w_query, w_key, w_value, w_out, b_out):
    global LAST_EXEC_NS, LAST_RESULTS
    x = np.asarray(x, dtype=np.float32)
    w_query = np.asarray(w_query, dtype=np.float32)
    w_key = np.asarray(w_key, dtype=np.float32)
    w_value = np.asarray(w_value, dtype=np.float32)
    w_out = np.asarray(w_out, dtype=np.float32)
    b_out = np.asarray(b_out, dtype=np.float32)

    nc = build()
    in_maps = prep_inputs(x, w_query, w_key, w_value, w_out, b_out)
    try:
        br = run_bass_kernel_spmd(nc, in_maps, list(range(NC)), trace=True)
    except Exception:
        br = run_bass_kernel_spmd(nc, in_maps, list(range(NC)), trace=False)
    LAST_EXEC_NS = br.exec_time_ns
    LAST_RESULTS = br

    out = np.empty((R, D), dtype=np.float32)
    for c in range(NC):
        out[c * 512:(c + 1) * 512, :] = br.results[c]["out"]
    return out.reshape(B, S, D)


if __name__ == "__main__":
    rng = np.random.default_rng(0)
    ins = {
        "x": rng.standard_normal((B, S, D), dtype=np.float32),
        "w_query": rng.standard_normal((D, D), dtype=np.float32) * 0.03,
        "w_key": rng.standard_normal((D, D), dtype=np.float32) * 0.03,
        "w_value": rng.standard_normal((D, D), dtype=np.float32) * 0.03,
        "w_out": rng.standard_normal((D, D), dtype=np.float32) * 0.03,
        "b_out": rng.standard_normal((D,), dtype=np.float32) * 0.03,
    }
    y = kernel(**ins)
    print("out", y.shape, y.dtype, "exec_ns", LAST_EXEC_NS)
